# revision 6
# baseline (speedup 1.0000x reference)
"""VQ codebook (nn_CodeBook) Trainium2 kernel.

Data-parallel over the N axis across 8 NeuronCores; codebook replicated.
Per core (NL = N/8 = 4096 rows):
  1. Normalize codebook rows (wn), store wn to DRAM scratch, and build
     transposed hi/lo bf16 operand tiles wnT[d][128, K] for the PE.
  2. For each m-tile of 128 rows: normalize x rows (xn), transpose to get
     xnT hi/lo bf16, compute dot = xn @ wn.T in 3 bf16 passes
     (hi*hi + hi*lo + lo*hi -> fp32-accuracy), argmax over K via per-chunk
     MAX8 + FIND_INDEX8, gather wn[idx] with indirect DMA, and accumulate
     the squared-error loss partial.
Host sums the 8 per-core loss partials and concatenates outputs.

argmin(dist) == argmax(dot) here because dist = ||xn||^2 + ||wn||^2 - 2 dot
with the row/codeword norm terms ~= 1 (verified exact on the fixed inputs).
"""

import numpy as np

N, D, K = 32768, 512, 8192
NCORES = 8
NL = N // NCORES  # rows per core
P = 128
KC = 1024         # dist chunk width (2 PSUM banks)
COMMITMENT_COST = 0.25

# matmul precision mode: "hilo" = 3-pass bf16 split (fp32-accuracy),
# "fp32" = native fp32 (4 cyc/row), "fp32r" = reduced-precision 1-pass.
MODE = "hilo"

_CACHE = {}


def _build(mode=MODE, nl=NL, k=K, d_dim=D, kc=KC):
    import concourse.bass as bass
    import concourse.tile as tile
    from concourse import bacc, mybir
    from concourse.masks import make_identity

    F32 = mybir.dt.float32
    BF16 = mybir.dt.bfloat16
    I32 = mybir.dt.int32
    U32 = mybir.dt.uint32
    AF = mybir.ActivationFunctionType
    ALU = mybir.AluOpType
    AX = mybir.AxisListType

    MT = nl // P       # m-tiles
    NKC = k // kc      # dist chunks per row
    DT = d_dim // P    # contraction sub-chunks
    WT = k // P        # codebook tiles

    nc = bacc.Bacc("TRN2", target_bir_lowering=False, debug=False)

    x_in = nc.dram_tensor("x", (nl, d_dim), F32, kind="ExternalInput").ap()
    cb_in = nc.dram_tensor("codebook", (k, d_dim), F32, kind="ExternalInput").ap()
    out_q = nc.dram_tensor("out_q", (nl, d_dim), F32, kind="ExternalOutput").ap()
    out_idx = nc.dram_tensor("out_idx", (nl, 1), I32, kind="ExternalOutput").ap()
    out_loss = nc.dram_tensor("out_loss", (P, 1), F32, kind="ExternalOutput").ap()
    wn_dram = nc.dram_tensor("wn_scratch", (k, d_dim), F32).ap()

    hilo = mode == "hilo"
    mm_dt = BF16 if hilo else (F32 if mode == "fp32" else mybir.dt.float32r)

    with tile.TileContext(nc) as tc:
        with (
            tc.tile_pool(name="const", bufs=1) as const_pool,
            tc.tile_pool(name="wnT", bufs=1) as wnT_pool,
            tc.tile_pool(name="wsetup", bufs=3) as wpool,
            tc.tile_pool(name="xio", bufs=3) as xpool,
            tc.tile_pool(name="xn", bufs=2) as xnpool,
            tc.tile_pool(name="xT", bufs=2) as xTpool,
            tc.tile_pool(name="scan", bufs=3) as scanpool,
            tc.tile_pool(name="merge", bufs=2) as mergepool,
            tc.tile_pool(name="qout", bufs=3) as qpool,
            tc.tile_pool(name="acc", bufs=1) as accpool,
            tc.tile_pool(name="psum_mm", bufs=3, space="PSUM") as psum_mm,
            tc.tile_pool(name="psum_tr", bufs=2, space="PSUM") as psum_tr,
        ):
            identity = const_pool.tile([P, P], F32)
            make_identity(nc, identity[:])

            loss_acc = accpool.tile([P, 1], F32)
            nc.vector.memset(loss_acc[:], 0.0)

            # Persistent transposed codebook operand tiles: wnT[d] is
            # [128(d-slice), K]; hi and lo parts for the bf16 split.
            wnT_hi = [wnT_pool.tile([P, k], mm_dt, tag=f"wnT_hi{d}", name=f"wnT_hi{d}") for d in range(DT)]
            wnT_lo = (
                [wnT_pool.tile([P, k], BF16, tag=f"wnT_lo{d}", name=f"wnT_lo{d}") for d in range(DT)]
                if hilo else None
            )

            # ---- codebook setup: normalize + transpose ----
            for t in range(WT):
                w_t = wpool.tile([P, d_dim], F32, tag="w_t")
                nc.sync.dma_start(out=w_t[:], in_=cb_in[t * P:(t + 1) * P, :])

                sq = wpool.tile([P, d_dim], F32, tag="sq")
                ssq = wpool.tile([P, 1], F32, tag="ssq")
                nc.scalar.activation(sq[:], w_t[:], AF.Square, accum_out=ssq[:])
                norm = wpool.tile([P, 1], F32, tag="norm")
                nc.scalar.activation(norm[:], ssq[:], AF.Sqrt)
                rec = wpool.tile([P, 1], F32, tag="rec")
                nc.vector.reciprocal(rec[:], norm[:])
                wn_t = wpool.tile([P, d_dim], F32, tag="wn_t")
                nc.scalar.activation(wn_t[:], w_t[:], AF.Copy, scale=rec[:, :1])
                nc.sync.dma_start(out=wn_dram[t * P:(t + 1) * P, :], in_=wn_t[:])

                # transpose the DT [128,128] blocks -> wnT[d][:, t*128:...]
                tr = psum_tr.tile([P, d_dim], F32, tag="tr")
                for d in range(DT):
                    nc.tensor.transpose(
                        out=tr[:, d * P:(d + 1) * P],
                        in_=wn_t[:, d * P:(d + 1) * P],
                        identity=identity[:],
                    )
                ks = slice(t * P, (t + 1) * P)
                for d in range(DT):
                    blk = tr[:, d * P:(d + 1) * P]
                    nc.scalar.activation(wnT_hi[d][:, ks], blk, AF.Copy)
                    if hilo:
                        nc.vector.tensor_tensor(
                            out=wnT_lo[d][:, ks], in0=blk,
                            in1=wnT_hi[d][:, ks], op=ALU.subtract,
                        )

            # ---- main loop over m-tiles ----
            for t in range(MT):
                ms = slice(t * P, (t + 1) * P)
                x_t = xpool.tile([P, d_dim], F32, tag="x_t")
                nc.sync.dma_start(out=x_t[:], in_=x_in[ms, :])

                sqx = xpool.tile([P, d_dim], F32, tag="sqx")
                ssqx = xpool.tile([P, 1], F32, tag="ssqx")
                nc.scalar.activation(sqx[:], x_t[:], AF.Square, accum_out=ssqx[:])
                normx = xpool.tile([P, 1], F32, tag="normx")
                nc.scalar.activation(normx[:], ssqx[:], AF.Sqrt)
                recx = xpool.tile([P, 1], F32, tag="recx")
                nc.vector.reciprocal(recx[:], normx[:])
                xn_t = xnpool.tile([P, d_dim], F32, tag="xn_t")
                nc.scalar.activation(xn_t[:], x_t[:], AF.Copy, scale=recx[:, :1])

                # transpose xn -> xnT (hi/lo bf16, or fp32/fp32r)
                trx = psum_tr.tile([P, d_dim], F32, tag="tr")
                for d in range(DT):
                    nc.tensor.transpose(
                        out=trx[:, d * P:(d + 1) * P],
                        in_=xn_t[:, d * P:(d + 1) * P],
                        identity=identity[:],
                    )
                xnT_hi = xTpool.tile([P, d_dim], mm_dt, tag="xnT_hi")
                xnT_lo = (
                    xTpool.tile([P, d_dim], BF16, tag="xnT_lo", name="xnT_lo")
                    if hilo else None
                )
                for d in range(DT):
                    blk = trx[:, d * P:(d + 1) * P]
                    dsl = slice(d * P, (d + 1) * P)
                    nc.scalar.activation(xnT_hi[:, dsl], blk, AF.Copy)
                    if hilo:
                        nc.vector.tensor_tensor(
                            out=xnT_lo[:, dsl], in0=blk,
                            in1=xnT_hi[:, dsl], op=ALU.subtract,
                        )

                # candidate buffers for the per-chunk argmax merge
                candv = mergepool.tile([P, NKC], F32, tag="candv")
                gidxf = mergepool.tile([P, NKC], F32, tag="gidxf")

                for c in range(NKC):
                    dist = psum_mm.tile([P, kc], F32, tag="dist")
                    for sub in range(kc // 512):
                        kbase = c * kc + sub * 512
                        osl = slice(sub * 512, (sub + 1) * 512)
                        rsl = slice(kbase, kbase + 512)
                        if hilo:
                            seq = []
                            for d in range(DT):
                                seq.append((xnT_hi, wnT_hi[d], d))
                                seq.append((xnT_hi, wnT_lo[d], d))
                                seq.append((xnT_lo, wnT_hi[d], d))
                        else:
                            seq = [(xnT_hi, wnT_hi[d], d) for d in range(DT)]
                        for i, (lhsrc, rsrc, d) in enumerate(seq):
                            nc.tensor.matmul(
                                out=dist[:, osl],
                                lhsT=lhsrc[:, d * P:(d + 1) * P],
                                rhs=rsrc[:, rsl],
                                start=(i == 0),
                                stop=(i == len(seq) - 1),
                            )
                    # scan: top-8 values + first-occurrence index of the max
                    mx8 = scanpool.tile([P, 8], F32, tag="mx8")
                    ix8 = scanpool.tile([P, 8], U32, tag="ix8")
                    nc.vector.max(out=mx8[:], in_=dist[:])
                    nc.vector.max_index(out=ix8[:], in_max=mx8[:], in_values=dist[:])
                    nc.vector.tensor_copy(candv[:, c:c + 1], mx8[:, 0:1])
                    ixf = scanpool.tile([P, 1], F32, tag="ixf")
                    nc.vector.tensor_copy(ixf[:], ix8[:, 0:1])
                    nc.vector.tensor_scalar_add(gidxf[:, c:c + 1], ixf[:], float(c * kc))

                # merge: smallest global index among chunks achieving the max
                vmax = mergepool.tile([P, 1], F32, tag="vmax")
                nc.vector.tensor_reduce(vmax[:], candv[:], axis=AX.X, op=ALU.max)
                notm = mergepool.tile([P, NKC], F32, tag="notm")
                nc.vector.tensor_tensor(
                    out=notm[:], in0=candv[:],
                    in1=vmax[:, 0:1].to_broadcast([P, NKC]), op=ALU.is_lt,
                )
                pen = mergepool.tile([P, NKC], F32, tag="pen")
                nc.vector.tensor_scalar(
                    out=pen[:], in0=notm[:], scalar1=float(2 * k), scalar2=None,
                    op0=ALU.mult,
                )
                nc.vector.tensor_tensor(out=pen[:], in0=pen[:], in1=gidxf[:], op=ALU.add)
                idxf = mergepool.tile([P, 1], F32, tag="idxf")
                nc.vector.tensor_reduce(idxf[:], pen[:], axis=AX.X, op=ALU.min)
                idx_i = mergepool.tile([P, 1], I32, tag="idx_i")
                nc.vector.tensor_copy(idx_i[:], idxf[:])
                nc.sync.dma_start(out=out_idx[ms, :], in_=idx_i[:])

                # gather quantized = wn[idx]
                q_t = qpool.tile([P, d_dim], F32, tag="q_t")
                nc.gpsimd.indirect_dma_start(
                    out=q_t[:],
                    out_offset=None,
                    in_=wn_dram[:],
                    in_offset=bass.IndirectOffsetOnAxis(ap=idx_i[:, :1], axis=0),
                )
                nc.sync.dma_start(out=out_q[ms, :], in_=q_t[:])

                # loss partial: sum((q - xn)^2) per row, accumulated
                diff = qpool.tile([P, d_dim], F32, tag="diff")
                nc.vector.tensor_tensor(out=diff[:], in0=q_t[:], in1=xn_t[:], op=ALU.subtract)
                dsq = qpool.tile([P, d_dim], F32, tag="dsq")
                lrow = qpool.tile([P, 1], F32, tag="lrow")
                nc.scalar.activation(dsq[:], diff[:], AF.Square, accum_out=lrow[:])
                nc.vector.tensor_tensor(out=loss_acc[:], in0=loss_acc[:], in1=lrow[:], op=ALU.add)

            nc.sync.dma_start(out=out_loss[:, :], in_=loss_acc[:])

    nc.finalize()
    return nc


def _run(x, codebook, trace=False, mode=MODE):
    from concourse.bass_utils import run_bass_kernel_spmd

    key = ("nc", mode)
    if key not in _CACHE:
        _CACHE[key] = _build(mode)
    nc = _CACHE[key]

    x = np.ascontiguousarray(np.asarray(x, dtype=np.float32))
    codebook = np.ascontiguousarray(np.asarray(codebook, dtype=np.float32))
    in_maps = [
        {"x": x[c * NL:(c + 1) * NL], "codebook": codebook}
        for c in range(NCORES)
    ]
    res = run_bass_kernel_spmd(nc, in_maps, core_ids=list(range(NCORES)), trace=trace)
    q = np.concatenate([res.results[c]["out_q"] for c in range(NCORES)], axis=0)
    idx = np.concatenate(
        [res.results[c]["out_idx"][:, 0] for c in range(NCORES)], axis=0
    ).astype(np.int32)
    loss_sum = sum(res.results[c]["out_loss"].sum(dtype=np.float64) for c in range(NCORES))
    loss = np.float32((1.0 + COMMITMENT_COST) * loss_sum / float(N * D))
    return (q, loss, idx), res


def kernel(x, codebook):
    (q, loss, idx), _ = _run(x, codebook)
    return q, loss, idx


# revision 10
# speedup vs baseline: 1.3886x; 1.3886x over previous
"""VQ codebook (nn_CodeBook) Trainium2 kernel.

Data-parallel over the N axis across 8 NeuronCores; codebook replicated.
Per core (NL = N/8 = 4096 rows):
  1. Normalize codebook rows (wn), store wn to DRAM scratch, and build
     transposed hi/lo bf16 operand tiles wnT[d][128, K] for the PE.
  2. For each m-tile of 128 rows: normalize x rows (xn), transpose to get
     xnT hi/lo bf16, compute dot = xn @ wn.T in 3 bf16 passes
     (hi*hi + hi*lo + lo*hi -> fp32-accuracy), argmax over K via per-chunk
     MAX8 + FIND_INDEX8, gather wn[idx] with indirect DMA, and accumulate
     the squared-error loss partial.
Host sums the 8 per-core loss partials and concatenates outputs.

argmin(dist) == argmax(dot) here because dist = ||xn||^2 + ||wn||^2 - 2 dot
with the row/codeword norm terms ~= 1 (verified exact on the fixed inputs).
"""

import numpy as np

N, D, K = 32768, 512, 8192
NCORES = 8
NL = N // NCORES  # rows per core
P = 128
KC = 1024         # dist chunk width (2 PSUM banks)
COMMITMENT_COST = 0.25

# matmul precision mode: "hilo" = 3-pass bf16 split (fp32-accuracy),
# "fp32" = native fp32 (4 cyc/row), "fp32r" = reduced-precision 1-pass.
MODE = "hilo"

_CACHE = {}


def _build(mode=MODE, nl=NL, k=K, d_dim=D, kc=KC):
    import concourse.bass as bass
    import concourse.tile as tile
    from concourse import bacc, mybir
    from concourse.masks import make_identity

    F32 = mybir.dt.float32
    BF16 = mybir.dt.bfloat16
    I32 = mybir.dt.int32
    U32 = mybir.dt.uint32
    AF = mybir.ActivationFunctionType
    ALU = mybir.AluOpType
    AX = mybir.AxisListType

    MT = nl // P       # m-tiles
    NKC = k // kc      # dist chunks per row
    DT = d_dim // P    # contraction sub-chunks
    WT = k // P        # codebook tiles

    nc = bacc.Bacc("TRN2", target_bir_lowering=False, debug=False)

    x_in = nc.dram_tensor("x", (nl, d_dim), F32, kind="ExternalInput").ap()
    cb_in = nc.dram_tensor("codebook", (k, d_dim), F32, kind="ExternalInput").ap()
    out_q = nc.dram_tensor("out_q", (nl, d_dim), F32, kind="ExternalOutput").ap()
    out_idx = nc.dram_tensor("out_idx", (nl, 1), I32, kind="ExternalOutput").ap()
    out_loss = nc.dram_tensor("out_loss", (P, 1), F32, kind="ExternalOutput").ap()
    wn_dram = nc.dram_tensor("wn_scratch", (k, d_dim), F32).ap()

    hilo = mode == "hilo"
    # fp32r tiles must be written as float32r so the producer rounds them
    # to the reduced-precision storage format the PE datapath expects.
    mm_dt = BF16 if hilo else (mybir.dt.float32r if mode == "fp32r" else F32)

    with tile.TileContext(nc) as tc:
        with (
            tc.tile_pool(name="const", bufs=1) as const_pool,
            tc.tile_pool(name="wnT", bufs=1) as wnT_pool,
            tc.tile_pool(name="wsetup", bufs=3) as wpool,
            tc.tile_pool(name="xio", bufs=3) as xpool,
            tc.tile_pool(name="xn", bufs=2) as xnpool,
            tc.tile_pool(name="xT", bufs=2) as xTpool,
            tc.tile_pool(name="scan", bufs=3) as scanpool,
            tc.tile_pool(name="merge", bufs=2) as mergepool,
            tc.tile_pool(name="qout", bufs=3) as qpool,
            tc.tile_pool(name="acc", bufs=1) as accpool,
            tc.tile_pool(name="psum_mm", bufs=3, space="PSUM") as psum_mm,
            tc.tile_pool(name="psum_tr", bufs=2, space="PSUM") as psum_tr,
        ):
            identity = const_pool.tile([P, P], F32)
            make_identity(nc, identity[:])

            loss_acc = accpool.tile([P, 1], F32)
            nc.vector.memset(loss_acc[:], 0.0)

            # Persistent transposed codebook operand tiles: wnT[d] is
            # [128(d-slice), K]; hi and lo parts for the bf16 split.
            wnT_hi = [wnT_pool.tile([P, k], mm_dt, tag=f"wnT_hi{d}", name=f"wnT_hi{d}") for d in range(DT)]
            wnT_lo = (
                [wnT_pool.tile([P, k], BF16, tag=f"wnT_lo{d}", name=f"wnT_lo{d}") for d in range(DT)]
                if hilo else None
            )

            # ---- codebook setup: normalize + transpose ----
            for t in range(WT):
                w_t = wpool.tile([P, d_dim], F32, tag="w_t")
                nc.sync.dma_start(out=w_t[:], in_=cb_in[t * P:(t + 1) * P, :])

                sq = wpool.tile([P, d_dim], F32, tag="sq")
                ssq = wpool.tile([P, 1], F32, tag="ssq")
                nc.scalar.activation(sq[:], w_t[:], AF.Square, accum_out=ssq[:])
                norm = wpool.tile([P, 1], F32, tag="norm")
                nc.scalar.activation(norm[:], ssq[:], AF.Sqrt)
                rec = wpool.tile([P, 1], F32, tag="rec")
                nc.vector.reciprocal(rec[:], norm[:])
                wn_t = wpool.tile([P, d_dim], F32, tag="wn_t")
                nc.scalar.activation(wn_t[:], w_t[:], AF.Copy, scale=rec[:, :1])
                nc.sync.dma_start(out=wn_dram[t * P:(t + 1) * P, :], in_=wn_t[:])

                # transpose the DT [128,128] blocks -> wnT[d][:, t*128:...]
                tr = psum_tr.tile([P, d_dim], F32, tag="tr")
                for d in range(DT):
                    nc.tensor.transpose(
                        out=tr[:, d * P:(d + 1) * P],
                        in_=wn_t[:, d * P:(d + 1) * P],
                        identity=identity[:],
                    )
                ks = slice(t * P, (t + 1) * P)
                for d in range(DT):
                    blk = tr[:, d * P:(d + 1) * P]
                    nc.scalar.activation(wnT_hi[d][:, ks], blk, AF.Copy)
                    if hilo:
                        nc.vector.tensor_tensor(
                            out=wnT_lo[d][:, ks], in0=blk,
                            in1=wnT_hi[d][:, ks], op=ALU.subtract,
                        )

            # ---- main loop over m-tiles ----
            for t in range(MT):
                ms = slice(t * P, (t + 1) * P)
                x_t = xpool.tile([P, d_dim], F32, tag="x_t")
                nc.sync.dma_start(out=x_t[:], in_=x_in[ms, :])

                sqx = xpool.tile([P, d_dim], F32, tag="sqx")
                ssqx = xpool.tile([P, 1], F32, tag="ssqx")
                nc.scalar.activation(sqx[:], x_t[:], AF.Square, accum_out=ssqx[:])
                normx = xpool.tile([P, 1], F32, tag="normx")
                nc.scalar.activation(normx[:], ssqx[:], AF.Sqrt)
                recx = xpool.tile([P, 1], F32, tag="recx")
                nc.vector.reciprocal(recx[:], normx[:])
                xn_t = xnpool.tile([P, d_dim], F32, tag="xn_t")
                nc.scalar.activation(xn_t[:], x_t[:], AF.Copy, scale=recx[:, :1])

                # transpose xn -> xnT (hi/lo bf16, or fp32/fp32r)
                trx = psum_tr.tile([P, d_dim], F32, tag="tr")
                for d in range(DT):
                    nc.tensor.transpose(
                        out=trx[:, d * P:(d + 1) * P],
                        in_=xn_t[:, d * P:(d + 1) * P],
                        identity=identity[:],
                    )
                xnT_hi = xTpool.tile([P, d_dim], mm_dt, tag="xnT_hi")
                xnT_lo = (
                    xTpool.tile([P, d_dim], BF16, tag="xnT_lo", name="xnT_lo")
                    if hilo else None
                )
                for d in range(DT):
                    blk = trx[:, d * P:(d + 1) * P]
                    dsl = slice(d * P, (d + 1) * P)
                    nc.scalar.activation(xnT_hi[:, dsl], blk, AF.Copy)
                    if hilo:
                        nc.vector.tensor_tensor(
                            out=xnT_lo[:, dsl], in0=blk,
                            in1=xnT_hi[:, dsl], op=ALU.subtract,
                        )

                # candidate buffers for the per-chunk argmax merge
                maxbuf = mergepool.tile([P, NKC * 8], F32, tag="maxbuf")
                gidxf = mergepool.tile([P, NKC], F32, tag="gidxf")

                for c in range(NKC):
                    dist = psum_mm.tile([P, kc], F32, tag="dist")
                    for sub in range(kc // 512):
                        kbase = c * kc + sub * 512
                        osl = slice(sub * 512, (sub + 1) * 512)
                        rsl = slice(kbase, kbase + 512)
                        if hilo:
                            seq = []
                            for d in range(DT):
                                seq.append((xnT_hi, wnT_hi[d], d))
                                seq.append((xnT_hi, wnT_lo[d], d))
                                seq.append((xnT_lo, wnT_hi[d], d))
                        else:
                            seq = [(xnT_hi, wnT_hi[d], d) for d in range(DT)]
                        for i, (lhsrc, rsrc, d) in enumerate(seq):
                            nc.tensor.matmul(
                                out=dist[:, osl],
                                lhsT=lhsrc[:, d * P:(d + 1) * P],
                                rhs=rsrc[:, rsl],
                                start=(i == 0),
                                stop=(i == len(seq) - 1),
                            )
                    # scan: top-8 values + first-occurrence index of the max
                    mx8 = maxbuf[:, 8 * c:8 * c + 8]
                    ix8 = scanpool.tile([P, 8], U32, tag="ix8")
                    nc.vector.max(out=mx8, in_=dist[:])
                    nc.vector.max_index(out=ix8[:], in_max=mx8, in_values=dist[:])
                    # uint32 index -> fp32 with the chunk base folded in
                    nc.vector.tensor_scalar(
                        out=gidxf[:, c:c + 1], in0=ix8[:, 0:1],
                        scalar1=float(c * kc), scalar2=None, op0=ALU.add,
                    )

                # merge: smallest global index among chunks achieving the max
                vmax = mergepool.tile([P, 1], F32, tag="vmax")
                nc.vector.tensor_reduce(vmax[:], maxbuf[:], axis=AX.X, op=ALU.max)
                candv = maxbuf[:].rearrange("p (c e) -> p c e", e=8)[:, :, 0:1]
                notm = mergepool.tile([P, NKC], F32, tag="notm")
                nc.vector.tensor_tensor(
                    out=notm[:], in0=candv,
                    in1=vmax[:, 0:1].to_broadcast([P, NKC, 1]), op=ALU.is_lt,
                )
                pen = mergepool.tile([P, NKC], F32, tag="pen")
                nc.vector.tensor_scalar(
                    out=pen[:], in0=notm[:], scalar1=float(2 * k), scalar2=None,
                    op0=ALU.mult,
                )
                nc.vector.tensor_tensor(out=pen[:], in0=pen[:], in1=gidxf[:], op=ALU.add)
                idxf = mergepool.tile([P, 1], F32, tag="idxf")
                nc.vector.tensor_reduce(idxf[:], pen[:], axis=AX.X, op=ALU.min)
                idx_i = mergepool.tile([P, 1], I32, tag="idx_i")
                nc.vector.tensor_copy(idx_i[:], idxf[:])
                nc.sync.dma_start(out=out_idx[ms, :], in_=idx_i[:])

                # gather quantized = wn[idx]
                q_t = qpool.tile([P, d_dim], F32, tag="q_t")
                nc.gpsimd.indirect_dma_start(
                    out=q_t[:],
                    out_offset=None,
                    in_=wn_dram[:],
                    in_offset=bass.IndirectOffsetOnAxis(ap=idx_i[:, :1], axis=0),
                )
                nc.sync.dma_start(out=out_q[ms, :], in_=q_t[:])

                # loss partial: sum((q - xn)^2) per row, accumulated
                diff = qpool.tile([P, d_dim], F32, tag="diff")
                nc.vector.tensor_tensor(out=diff[:], in0=q_t[:], in1=xn_t[:], op=ALU.subtract)
                dsq = qpool.tile([P, d_dim], F32, tag="dsq")
                lrow = qpool.tile([P, 1], F32, tag="lrow")
                nc.scalar.activation(dsq[:], diff[:], AF.Square, accum_out=lrow[:])
                nc.vector.tensor_tensor(out=loss_acc[:], in0=loss_acc[:], in1=lrow[:], op=ALU.add)

            nc.sync.dma_start(out=out_loss[:, :], in_=loss_acc[:])

    nc.finalize()
    return nc


def _run(x, codebook, trace=False, mode=MODE):
    from concourse.bass_utils import run_bass_kernel_spmd

    key = ("nc", mode)
    if key not in _CACHE:
        _CACHE[key] = _build(mode)
    nc = _CACHE[key]

    x = np.ascontiguousarray(np.asarray(x, dtype=np.float32))
    codebook = np.ascontiguousarray(np.asarray(codebook, dtype=np.float32))
    in_maps = [
        {"x": x[c * NL:(c + 1) * NL], "codebook": codebook}
        for c in range(NCORES)
    ]
    res = run_bass_kernel_spmd(nc, in_maps, core_ids=list(range(NCORES)), trace=trace)
    q = np.concatenate([res.results[c]["out_q"] for c in range(NCORES)], axis=0)
    idx = np.concatenate(
        [res.results[c]["out_idx"][:, 0] for c in range(NCORES)], axis=0
    ).astype(np.int32)
    loss_sum = sum(res.results[c]["out_loss"].sum(dtype=np.float64) for c in range(NCORES))
    loss = np.float32((1.0 + COMMITMENT_COST) * loss_sum / float(N * D))
    return (q, loss, idx), res


def kernel(x, codebook):
    (q, loss, idx), _ = _run(x, codebook)
    return q, loss, idx
